# revision 5
# baseline (speedup 1.0000x reference)
"""
Trainium2 Bass kernel for MultiHeadSelfAttention with RoPE (causal).

Reference semantics (fp32):
  q/k/v = x @ w{q,k,v}.T  (w layout [d_out, d_in])
  split into 16 heads of dh=64, interleaved-pair RoPE on q, k
  causal softmax attention, merge heads, out = attn_out @ wo.T

Sharding: 8 cores = 2 (batch) x 4 (head groups of 4 heads).
Each core computes a partial output [2048, 1024] = O_heads @ wo[:, cols].T;
host sums the 4 head-group partials per batch element.

Device-side layout (per core, hg = head group, b = batch):
  xT   [1024, 2048]  = x[b].T
  wqT  [1024, 256]   = wq[hg*256:(hg+1)*256, :].T   (same wkT, wvT)
  woT  [256, 1024]   = wo[:, hg*256:(hg+1)*256].T
  cosT [128, 2048]   cosT[d, t] = cos(t * theta^(-2*((d%64)//2)/64))
  sinT [128, 2048]   likewise sin
  rmat [128, 128]    = R.T where rot = R @ q is the interleaved pair rotation
  out  [2048, 1024]  partial output (needs sum over the 4 head groups)

Pipeline on device:
  1. QT/KT/VT = w.T-chunks (stationary) x xT (moving) -> [256, 2048] each,
     RoPE applied to QT/KT via rot = R@QT (PE matmul) + elementwise tables,
     VT transposed back to seq-major V [2048, 256] via PE transposes.
  2. Per (head, 128-row tile i): S = QT_h.T @ KT_h chunks (causal, chunked
     by 512 cols), additive causal mask on the diagonal chunk, exp (ACT,
     scale=1/8, accum row sums), normalize P by 1/rowsum (gpsimd),
     PE-transpose P blocks into PT buffer.
  3. O^T accumulation per (head, 512-col chunk): O^T = V.T @ P^T.
  4. Final projection out[seq-tile] = (O^T chunks).T @ woT chunks.
"""

import math
import sys

import numpy as np

sys.path.insert(0, "/opt/trn_rl_repo")

import concourse.bass as bass  # noqa: E402
import concourse.mybir as mybir  # noqa: E402
import concourse.tile as tile  # noqa: E402
from concourse import bacc  # noqa: E402
from concourse.masks import make_identity  # noqa: E402

F32 = mybir.dt.float32

D_MODEL = 1024
NUM_HEADS = 16
DH = 64
THETA = 10000.0
SEQ = 2048
BATCH = 2
N_CORES = 8
HPC = 4  # heads per core
DQ = HPC * DH  # 256 projection dims per core
NT = SEQ // 128  # 16 seq tiles
NCH = SEQ // 512  # 4 seq chunks
MASK_VAL = -1.0e6

# Matmul dtype knob: float32 (exact, 4 cyc/col) or float32r (~1 cyc/col at
# N>=256, reduced precision).
import os as _os
USE_F32R = _os.environ.get("MHSA_F32R", "1") == "1"
# dtype used for matmul operands (DRAM + SBUF); float32r is bit-identical
# to float32 on the host side but runs the PE at ~4x fp32 rate.
MM = mybir.dt.float32r if USE_F32R else F32


def emit(nc):
    """Emit the per-core kernel IR. Same program for all 8 cores (SPMD)."""
    xT = nc.dram_tensor("xT", [D_MODEL, SEQ], MM, kind="ExternalInput")
    wqT = nc.dram_tensor("wqT", [D_MODEL, DQ], MM, kind="ExternalInput")
    wkT = nc.dram_tensor("wkT", [D_MODEL, DQ], MM, kind="ExternalInput")
    wvT = nc.dram_tensor("wvT", [D_MODEL, DQ], MM, kind="ExternalInput")
    woT = nc.dram_tensor("woT", [DQ, D_MODEL], MM, kind="ExternalInput")
    cosT = nc.dram_tensor("cosT", [128, SEQ], F32, kind="ExternalInput")
    sinT = nc.dram_tensor("sinT", [128, SEQ], F32, kind="ExternalInput")
    rmat = nc.dram_tensor("rmat", [128, 128], F32, kind="ExternalInput")
    out = nc.dram_tensor("out", [SEQ, D_MODEL], F32, kind="ExternalOutput")

    ExpF = mybir.ActivationFunctionType.Exp
    X = mybir.AxisListType.X

    with tile.TileContext(nc) as tc:
        with tc.tile_pool(name="persist", bufs=1) as persist:
            # --- persistent SBUF tensors ---
            wq_sb = persist.tile([128, 8, DQ], MM, tag="wq")
            wk_sb = persist.tile([128, 8, DQ], MM, tag="wk")
            wv_sb = persist.tile([128, 8, DQ], MM, tag="wv")
            nc.sync.dma_start(wq_sb, wqT[:, :].rearrange("(k p) m -> p k m", p=128))
            nc.sync.dma_start(wk_sb, wkT[:, :].rearrange("(k p) m -> p k m", p=128))
            nc.sync.dma_start(wv_sb, wvT[:, :].rearrange("(k p) m -> p k m", p=128))
            wo_sb = persist.tile([128, 2, D_MODEL], MM, tag="wo")
            nc.sync.dma_start(wo_sb, woT[:, :].rearrange("(k p) e -> p k e", p=128))
            rm_sb = persist.tile([128, 128], F32, tag="rm")
            nc.sync.dma_start(rm_sb, rmat[:, :])
            ident = persist.tile([128, 128], F32, tag="ident")
            make_identity(nc, ident)
            # Causal mask master: maskM[p, u] = 0 if u <= p + 384 else MASK_VAL.
            # Slice [384-128*ir : 896-128*ir] gives the mask for diagonal
            # chunk of row tile with i % 4 == ir.
            maskM = persist.tile([128, 896], F32, tag="maskM")
            nc.gpsimd.memset(maskM, 0.0)
            nc.gpsimd.affine_select(
                out=maskM,
                in_=maskM,
                compare_op=mybir.AluOpType.is_ge,
                fill=MASK_VAL,
                base=384,
                pattern=[[-1, 896]],
                channel_multiplier=1,
            )
            zblk = persist.tile([128, 128], F32, tag="zblk")
            nc.vector.memset(zblk, 0.0)
            QT = persist.tile([128, 2, SEQ], MM, tag="QT")
            KT = persist.tile([128, 2, SEQ], MM, tag="KT")
            V = persist.tile([128, NT, DQ], MM, tag="V")
            OT = persist.tile([128, 2, SEQ], MM, tag="OT")

            # ---------- Phase 1: projections + RoPE ----------
            with tc.tile_pool(name="p1", bufs=2) as p1pool, \
                 tc.tile_pool(name="p1s", bufs=3) as p1s, \
                 tc.tile_pool(name="cs", bufs=1) as cspool, \
                 tc.tile_pool(name="ps1", bufs=4, space=bass.MemorySpace.PSUM) as ps1, \
                 tc.tile_pool(name="psrot", bufs=2, space=bass.MemorySpace.PSUM) as psrot, \
                 tc.tile_pool(name="psvt", bufs=2, space=bass.MemorySpace.PSUM) as psvt:
                cos_sb = cspool.tile([128, SEQ], F32, tag="cos")
                sin_sb = cspool.tile([128, SEQ], F32, tag="sin")
                nc.sync.dma_start(cos_sb, cosT[:, :])
                nc.sync.dma_start(sin_sb, sinT[:, :])

                xTr = xT[:, :].rearrange("(k p) s -> p k s", p=128)
                for n in range(NCH):
                    xsl = p1pool.tile([128, 8, 512], MM, tag="xslab")
                    nc.sync.dma_start(xsl, xTr[:, :, n * 512:(n + 1) * 512])
                    for t, w_sb in enumerate((wq_sb, wk_sb, wv_sb)):
                        for mp in range(2):
                            pt = ps1.tile([128, 512], F32, tag="proj")
                            for k in range(8):
                                nc.tensor.matmul(
                                    pt,
                                    w_sb[:, k, mp * 128:(mp + 1) * 128],
                                    xsl[:, k, :],
                                    start=(k == 0),
                                    stop=(k == 7),
                                )
                            if t < 2:
                                # RoPE: dst = raw * cos + (R @ raw) * sin
                                raw = p1s.tile([128, 512], F32, tag="raw")
                                nc.vector.tensor_copy(raw, pt)
                                rp = psrot.tile([128, 512], F32, tag="rotp")
                                # rot matmul kept in full f32 (exact: R is +-1)
                                nc.tensor.matmul(rp, rm_sb, raw, start=True, stop=True)
                                rot = p1s.tile([128, 512], F32, tag="rots")
                                nc.vector.tensor_copy(rot, rp)
                                dstT = QT if t == 0 else KT
                                dst = dstT[:, mp, n * 512:(n + 1) * 512]
                                cs = cos_sb[:, n * 512:(n + 1) * 512]
                                sn = sin_sb[:, n * 512:(n + 1) * 512]
                                nc.vector.tensor_mul(dst, raw, cs)
                                nc.vector.tensor_mul(rot, rot, sn)
                                nc.vector.tensor_add(dst, dst, rot)
                            else:
                                # V: transpose back to seq-major
                                vtr = p1s.tile([128, 512], F32, tag="vtr")
                                nc.vector.tensor_copy(vtr, pt)
                                for sub in range(4):
                                    tp = psvt.tile([128, 128], F32, tag="vtp")
                                    nc.tensor.transpose(
                                        tp, vtr[:, sub * 128:(sub + 1) * 128], ident
                                    )
                                    nc.vector.tensor_copy(
                                        V[:, n * 4 + sub, mp * 128:(mp + 1) * 128], tp
                                    )

            # ---------- Phase 2: attention + output projection ----------
            with tc.tile_pool(name="p2", bufs=2) as p2pool, \
                 tc.tile_pool(name="ptp", bufs=1) as ptpool, \
                 tc.tile_pool(name="sm", bufs=4) as smpool, \
                 tc.tile_pool(name="outp", bufs=2) as outpool, \
                 tc.tile_pool(name="ps_s", bufs=3, space=bass.MemorySpace.PSUM) as ps_s, \
                 tc.tile_pool(name="ps_t", bufs=2, space=bass.MemorySpace.PSUM) as ps_t, \
                 tc.tile_pool(name="ps_o", bufs=2, space=bass.MemorySpace.PSUM) as ps_o, \
                 tc.tile_pool(name="ps_p", bufs=1, space=bass.MemorySpace.PSUM) as ps_p:
                for c in range(NCH):
                    for h in range(HPC):
                        pt_h = h // 2
                        po = 64 * (h % 2)
                        ptb = ptpool.tile([128, NT, 512], MM, tag="ptb")
                        for ir in range(4):
                            i = 4 * c + ir
                            Pb = p2pool.tile([128, SEQ], F32, tag="P")
                            sums = smpool.tile([128, 4], F32, tag="sums")
                            for j2 in range(c + 1):
                                sp = ps_s.tile([128, 512], F32, tag="s")
                                nc.tensor.matmul(
                                    sp,
                                    QT[po:po + 64, pt_h, i * 128:(i + 1) * 128],
                                    KT[po:po + 64, pt_h, j2 * 512:(j2 + 1) * 512],
                                    start=True,
                                    stop=True,
                                )
                                if j2 == c:
                                    nc.vector.tensor_add(
                                        sp, sp,
                                        maskM[:, 384 - 128 * ir: 896 - 128 * ir],
                                    )
                                nc.scalar.activation(
                                    Pb[:, j2 * 512:(j2 + 1) * 512],
                                    sp,
                                    ExpF,
                                    scale=1.0 / math.sqrt(DH),
                                    accum_out=sums[:, j2:j2 + 1],
                                )
                            tot = smpool.tile([128, 1], F32, tag="tot")
                            nc.vector.reduce_sum(tot, sums[:, :c + 1], axis=X)
                            nc.vector.reciprocal(tot, tot)
                            nc.gpsimd.tensor_scalar_mul(
                                Pb[:, :(i + 1) * 128], Pb[:, :(i + 1) * 128], tot
                            )
                            for j in range(i + 1):
                                tp = ps_t.tile([128, 128], F32, tag="tp")
                                nc.tensor.transpose(
                                    tp, Pb[:, j * 128:(j + 1) * 128], ident
                                )
                                nc.vector.tensor_copy(
                                    ptb[:, j, ir * 128:(ir + 1) * 128], tp
                                )
                        # zero the blocks above the diagonal inside this chunk
                        for ir in range(4):
                            for jr in range(ir + 1, 4):
                                nc.vector.tensor_copy(
                                    ptb[:, 4 * c + jr, ir * 128:(ir + 1) * 128], zblk
                                )
                        # O^T accumulation for this (head, chunk)
                        op = ps_o.tile([64, 512], F32, tag="ot")
                        nj = 4 * c + 4
                        for j in range(nj):
                            nc.tensor.matmul(
                                op,
                                V[:, j, h * 64:(h + 1) * 64],
                                ptb[:, j, :],
                                start=(j == 0),
                                stop=(j == nj - 1),
                            )
                        nc.vector.tensor_copy(
                            OT[po:po + 64, pt_h, c * 512:(c + 1) * 512], op
                        )
                    # output projection for the 4 row tiles of this chunk
                    for ir in range(4):
                        i = 4 * c + ir
                        ob = outpool.tile([128, D_MODEL], F32, tag="ob")
                        for nn2 in range(2):
                            pp = ps_p.tile([128, 512], F32, tag="pp")
                            for kk in range(2):
                                nc.tensor.matmul(
                                    pp,
                                    OT[:, kk, i * 128:(i + 1) * 128],
                                    wo_sb[:, kk, nn2 * 512:(nn2 + 1) * 512],
                                    start=(kk == 0),
                                    stop=(kk == 1),
                                )
                            nc.scalar.copy(ob[:, nn2 * 512:(nn2 + 1) * 512], pp)
                        nc.sync.dma_start(out[i * 128:(i + 1) * 128, :], ob)
    return nc


def build():
    nc = bacc.Bacc("TRN2", target_bir_lowering=False, debug=False)
    emit(nc)
    nc.compile()
    return nc


def host_tables():
    """cosT/sinT [128, 2048] and rmat [128, 128] (same for every core)."""
    d = np.arange(128)
    pair = (d % DH) // 2  # pair index within a head
    inv_freq = THETA ** (-2.0 * pair / DH)  # [128]
    t = np.arange(SEQ, dtype=np.float64)
    ang = t[None, :] * inv_freq[:, None]  # [128, 2048]
    cosT = np.cos(ang).astype(np.float32)
    sinT = np.sin(ang).astype(np.float32)
    R = np.zeros((128, 128), dtype=np.float32)
    idx = np.arange(0, 128, 2)
    R[idx, idx + 1] = -1.0
    R[idx + 1, idx] = 1.0
    rmat = np.ascontiguousarray(R.T)
    return cosT, sinT, rmat


def make_in_maps(x, wq, wk, wv, wo):
    x = np.asarray(x, dtype=np.float32)
    wq = np.asarray(wq, dtype=np.float32)
    wk = np.asarray(wk, dtype=np.float32)
    wv = np.asarray(wv, dtype=np.float32)
    wo = np.asarray(wo, dtype=np.float32)
    cosT, sinT, rmat = host_tables()
    in_maps = []
    for core in range(N_CORES):
        b, hg = core // 4, core % 4
        rows = slice(hg * DQ, (hg + 1) * DQ)
        in_maps.append({
            "xT": np.ascontiguousarray(x[b].T),
            "wqT": np.ascontiguousarray(wq[rows, :].T),
            "wkT": np.ascontiguousarray(wk[rows, :].T),
            "wvT": np.ascontiguousarray(wv[rows, :].T),
            "woT": np.ascontiguousarray(wo[:, rows].T),
            "cosT": cosT,
            "sinT": sinT,
            "rmat": rmat,
        })
    return in_maps


_NC = None
LAST_RESULTS = None


def kernel(x, wq, wk, wv, wo, token_positions=None, **_kwargs):
    """Full-input, full-output entry point. Distributes over 8 NeuronCores."""
    global _NC, LAST_RESULTS
    from concourse import bass_utils

    if _NC is None:
        _NC = build()
    in_maps = make_in_maps(x, wq, wk, wv, wo)
    res = bass_utils.run_bass_kernel_spmd(
        _NC, in_maps, core_ids=list(range(N_CORES))
    )
    LAST_RESULTS = res
    outs = [np.asarray(r["out"]) for r in res.results]
    full = np.empty((BATCH, SEQ, D_MODEL), dtype=np.float32)
    for b in range(BATCH):
        full[b] = outs[4 * b] + outs[4 * b + 1] + outs[4 * b + 2] + outs[4 * b + 3]
    return full


# revision 13
# speedup vs baseline: 2.5659x; 2.5659x over previous
"""
Trainium2 Bass kernel for MultiHeadSelfAttention with RoPE (causal).

Reference semantics (fp32):
  q/k/v = x @ w{q,k,v}.T  (w layout [d_out, d_in])
  split into 16 heads of dh=64, interleaved-pair RoPE on q, k
  causal softmax attention, merge heads, out = attn_out @ wo.T

Sharding: 8 cores = 2 (batch) x 4 (head groups of 4 heads).
Each core computes a partial output [2048, 1024] = O_heads @ wo[:, cols].T;
host sums the 4 head-group partials per batch element.

Device-side layout (per core, hg = head group, b = batch):
  xT   [1024, 2048]  = x[b].T
  wqT  [1024, 256]   = wq[hg*256:(hg+1)*256, :].T   (same wkT, wvT)
  woT  [256, 1024]   = wo[:, hg*256:(hg+1)*256].T
  cosT [128, 2048]   cosT[d, t] = cos(t * theta^(-2*((d%64)//2)/64))
  sinT [128, 2048]   likewise sin
  rmat [128, 128]    = R.T where rot = R @ q is the interleaved pair rotation
  out  [2048, 1024]  partial output (needs sum over the 4 head groups)

Pipeline on device:
  1. QT/KT/VT = w.T-chunks (stationary) x xT (moving) -> [256, 2048] each,
     RoPE applied to QT/KT via rot = R@QT (PE matmul) + elementwise tables,
     VT transposed back to seq-major V [2048, 256] via PE transposes.
  2. Per (head, 128-row tile i): S = QT_h.T @ KT_h chunks (causal, chunked
     by 512 cols), additive causal mask on the diagonal chunk, exp (ACT,
     scale=1/8, accum row sums), normalize P by 1/rowsum (gpsimd),
     PE-transpose P blocks into PT buffer.
  3. O^T accumulation per (head, 512-col chunk): O^T = V.T @ P^T.
  4. Final projection out[seq-tile] = (O^T chunks).T @ woT chunks.
"""

import math
import sys

import numpy as np

sys.path.insert(0, "/opt/trn_rl_repo")

import concourse.bass as bass  # noqa: E402
import concourse.mybir as mybir  # noqa: E402
import concourse.tile as tile  # noqa: E402
from concourse import bacc  # noqa: E402
from concourse.masks import make_identity  # noqa: E402

F32 = mybir.dt.float32

D_MODEL = 1024
NUM_HEADS = 16
DH = 64
THETA = 10000.0
SEQ = 2048
BATCH = 2
N_CORES = 8
HPC = 4  # heads per core
DQ = HPC * DH  # 256 projection dims per core
NT = SEQ // 128  # 16 seq tiles
NCH = SEQ // 512  # 4 seq chunks
MASK_VAL = -1.0e6

# Matmul dtype knob: float32 (exact, 4 cyc/col) or float32r (~1 cyc/col at
# N>=256, reduced precision).
import os as _os
USE_F32R = _os.environ.get("MHSA_F32R", "1") == "1"
REPEAT = int(_os.environ.get("MHSA_REPEAT", "1"))
# dtype used for matmul operands (DRAM + SBUF); float32r is bit-identical
# to float32 on the host side but runs the PE at ~4x fp32 rate.
MM = mybir.dt.float32r if USE_F32R else F32


def emit(nc):
    """Emit the per-core kernel IR. Same program for all 8 cores (SPMD)."""
    xT = nc.dram_tensor("xT", [D_MODEL, SEQ], MM, kind="ExternalInput")
    wqT = nc.dram_tensor("wqT", [D_MODEL, DQ], MM, kind="ExternalInput")
    wkT = nc.dram_tensor("wkT", [D_MODEL, DQ], MM, kind="ExternalInput")
    wvT = nc.dram_tensor("wvT", [D_MODEL, DQ], MM, kind="ExternalInput")
    woT = nc.dram_tensor("woT", [DQ, D_MODEL], MM, kind="ExternalInput")
    cosT = nc.dram_tensor("cosT", [128, SEQ], F32, kind="ExternalInput")
    sinT = nc.dram_tensor("sinT", [128, SEQ], F32, kind="ExternalInput")
    rmat = nc.dram_tensor("rmat", [128, 128], MM, kind="ExternalInput")
    out = nc.dram_tensor("out", [SEQ, D_MODEL], F32, kind="ExternalOutput")

    ExpF = mybir.ActivationFunctionType.Exp
    X = mybir.AxisListType.X

    with tile.TileContext(nc) as tc:
        with tc.tile_pool(name="persist", bufs=1) as persist:
          for _rep in range(REPEAT):
            # --- persistent SBUF tensors ---
            wq_sb = persist.tile([128, 8, DQ], MM, tag="wq")
            wk_sb = persist.tile([128, 8, DQ], MM, tag="wk")
            wv_sb = persist.tile([128, 8, DQ], MM, tag="wv")
            nc.sync.dma_start(wq_sb, wqT[:, :].rearrange("(k p) m -> p k m", p=128))
            nc.sync.dma_start(wk_sb, wkT[:, :].rearrange("(k p) m -> p k m", p=128))
            nc.sync.dma_start(wv_sb, wvT[:, :].rearrange("(k p) m -> p k m", p=128))
            wo_sb = persist.tile([128, 2, D_MODEL], MM, tag="wo")
            nc.sync.dma_start(wo_sb, woT[:, :].rearrange("(k p) e -> p k e", p=128))
            rm_sb = persist.tile([128, 128], MM, tag="rm")
            nc.sync.dma_start(rm_sb, rmat[:, :])
            ident = persist.tile([128, 128], F32, tag="ident")
            make_identity(nc, ident)
            ones_f32 = persist.tile([128, 1], F32, tag="ones_f32")
            nc.vector.memset(ones_f32, 1.0)
            ones_mm = persist.tile([1, 64], MM, tag="ones_mm")
            nc.vector.tensor_copy(ones_mm, ones_f32[0:1, 0:1].to_broadcast([1, 64]))
            QT = persist.tile([128, 2, SEQ], MM, tag="QT")
            KT = persist.tile([128, 2, SEQ], MM, tag="KT")
            # V2: per head 65 columns: 64 value dims + a ones column that
            # makes the O^T matmul also produce the softmax row-sums in
            # output partition 64.
            V2 = persist.tile([128, NT, HPC * 65], MM, tag="V2")
            OT = persist.tile([128, 2, SEQ], MM, tag="OT")
            # fill the ones columns of V2
            for h in range(HPC):
                nc.vector.tensor_copy(
                    V2[:, :, h * 65 + 64:h * 65 + 65],
                    ones_f32.to_broadcast([128, NT, 1]),
                )

            # ---------- Phase 1: projections + RoPE ----------
            with tc.tile_pool(name="p1", bufs=2) as p1pool, \
                 tc.tile_pool(name="p1s", bufs=3) as p1s, \
                 tc.tile_pool(name="cs", bufs=1) as cspool, \
                 tc.tile_pool(name="ps1", bufs=4, space=bass.MemorySpace.PSUM) as ps1, \
                 tc.tile_pool(name="psrot", bufs=2, space=bass.MemorySpace.PSUM) as psrot, \
                 tc.tile_pool(name="psvt", bufs=2, space=bass.MemorySpace.PSUM) as psvt:
                cos_sb = cspool.tile([128, SEQ], F32, tag="cos")
                sin_sb = cspool.tile([128, SEQ], F32, tag="sin")
                nc.sync.dma_start(cos_sb, cosT[:, :])
                nc.sync.dma_start(sin_sb, sinT[:, :])

                xTr = xT[:, :].rearrange("(k p) s -> p k s", p=128)
                for n in range(NCH):
                    xsl = p1pool.tile([128, 8, 512], MM, tag="xslab")
                    nc.sync.dma_start(xsl, xTr[:, :, n * 512:(n + 1) * 512])
                    for t, w_sb in enumerate((wq_sb, wk_sb, wv_sb)):
                        for mp in range(2):
                            pt = ps1.tile([128, 512], F32, tag="proj")
                            for k in range(8):
                                nc.tensor.matmul(
                                    pt,
                                    w_sb[:, k, mp * 128:(mp + 1) * 128],
                                    xsl[:, k, :],
                                    start=(k == 0),
                                    stop=(k == 7),
                                )
                            if t < 2:
                                # RoPE: dst = raw * cos + (R @ raw) * sin
                                raw = p1s.tile([128, 512], MM, tag="raw")
                                nc.scalar.copy(raw, pt)
                                rp = psrot.tile([128, 512], F32, tag="rotp")
                                nc.tensor.matmul(rp, rm_sb, raw, start=True, stop=True)
                                rot = p1s.tile([128, 512], F32, tag="rots")
                                nc.scalar.copy(rot, rp)
                                dstT = QT if t == 0 else KT
                                dst = dstT[:, mp, n * 512:(n + 1) * 512]
                                cs = cos_sb[:, n * 512:(n + 1) * 512]
                                sn = sin_sb[:, n * 512:(n + 1) * 512]
                                nc.vector.tensor_mul(dst, raw, cs)
                                nc.vector.tensor_mul(rot, rot, sn)
                                nc.vector.tensor_add(dst, dst, rot)
                            else:
                                # V: transpose back to seq-major, split into
                                # the two heads' 65-column slots of V2
                                vtr = p1s.tile([128, 512], F32, tag="vtr")
                                nc.vector.tensor_copy(vtr, pt)
                                for sub in range(4):
                                    tp = psvt.tile([128, 128], F32, tag="vtp")
                                    nc.tensor.transpose(
                                        tp, vtr[:, sub * 128:(sub + 1) * 128], ident
                                    )
                                    a = 2 * mp * 65
                                    dst2 = V2[:, n * 4 + sub, a:a + 130].rearrange(
                                        "p (h w) -> p h w", h=2)[:, :, 0:64]
                                    src2 = tp[:, :].rearrange(
                                        "p (h w) -> p h w", h=2)
                                    nc.vector.tensor_copy(dst2, src2)

            # ---------- Phase 2: attention + output projection ----------
            # Per (head h, query chunk c of 512): for each key tile j <=
            # 4c+3, compute S^T[j-tile, c-chunk] = K_j @ Q_c^T, mask the
            # diagonal tile, exp -> P^T straight into the ptb buffer, then
            # O^T[:, c] = sum_j [V_j | 1]^T @ P^T_j (65 rows: 64 output dims
            # + softmax row-sums), normalize at eviction.
            with tc.tile_pool(name="ptp", bufs=2) as ptpool, \
                 tc.tile_pool(name="sm", bufs=4) as smpool, \
                 tc.tile_pool(name="outp", bufs=2) as outpool, \
                 tc.tile_pool(name="ps_s", bufs=2, space=bass.MemorySpace.PSUM) as ps_s, \
                 tc.tile_pool(name="ps_o", bufs=2, space=bass.MemorySpace.PSUM) as ps_o, \
                 tc.tile_pool(name="ps_b", bufs=1, space=bass.MemorySpace.PSUM) as ps_b, \
                 tc.tile_pool(name="ps_p", bufs=1, space=bass.MemorySpace.PSUM) as ps_p:
                for c in range(NCH):
                    for h in range(HPC):
                        pt_h = h // 2
                        po = 64 * (h % 2)
                        ptb = ptpool.tile([128, NT, 512], MM, tag="ptb")
                        nj = 4 * c + 4
                        for jj in range(0, nj, 2):
                            sp = ps_s.tile([128, 1024], F32, tag="s")
                            for u in range(2):
                                j = jj + u
                                nc.tensor.matmul(
                                    sp[:, u * 512:(u + 1) * 512],
                                    KT[po:po + 64, pt_h, j * 128:(j + 1) * 128],
                                    QT[po:po + 64, pt_h, c * 512:(c + 1) * 512],
                                    start=True,
                                    stop=True,
                                )
                            nc.scalar.activation(
                                ptb[:, jj:jj + 2, :],
                                sp[:, :].rearrange("p (a b) -> p a b", a=2),
                                ExpF,
                                scale=1.0 / math.sqrt(DH),
                            )
                            for u in range(2):
                                j = jj + u
                                if j // 4 == c:
                                    # zero the sq < sk (above-diagonal) region
                                    nc.gpsimd.affine_select(
                                        out=ptb[:, j, :],
                                        in_=ptb[:, j, :],
                                        compare_op=mybir.AluOpType.is_ge,
                                        fill=0.0,
                                        base=-128 * (j % 4),
                                        pattern=[[1, 512]],
                                        channel_multiplier=-1,
                                    )
                        # O^T accumulation; row 64 = softmax row sums
                        op = ps_o.tile([65, 512], F32, tag="ot")
                        for j in range(nj):
                            nc.tensor.matmul(
                                op,
                                V2[:, j, h * 65:(h + 1) * 65],
                                ptb[:, j, :],
                                start=(j == 0),
                                stop=(j == nj - 1),
                            )
                        # normalize: recip of row sums, broadcast to 64
                        # partitions via a K=1 matmul, multiply at eviction
                        rec = smpool.tile([1, 512], MM, tag="rec")
                        with nc.allow_low_precision(reason="softmax recip bcast"):
                            nc.vector.reciprocal(rec, op[64:65, :])
                        bc = ps_b.tile([64, 512], F32, tag="bc")
                        nc.tensor.matmul(bc, ones_mm, rec, start=True, stop=True)
                        bcs = smpool.tile([64, 512], F32, tag="bcs")
                        nc.vector.tensor_copy(bcs, bc)
                        nc.vector.tensor_mul(
                            OT[po:po + 64, pt_h, c * 512:(c + 1) * 512],
                            op[0:64, :], bcs,
                        )
                    # output projection for the 4 row tiles of this chunk
                    for ir in range(4):
                        i = 4 * c + ir
                        ob = outpool.tile([128, D_MODEL], F32, tag="ob")
                        for nn2 in range(2):
                            pp = ps_p.tile([128, 512], F32, tag="pp")
                            for kk in range(2):
                                nc.tensor.matmul(
                                    pp,
                                    OT[:, kk, i * 128:(i + 1) * 128],
                                    wo_sb[:, kk, nn2 * 512:(nn2 + 1) * 512],
                                    start=(kk == 0),
                                    stop=(kk == 1),
                                )
                            nc.vector.tensor_copy(ob[:, nn2 * 512:(nn2 + 1) * 512], pp)
                        nc.sync.dma_start(out[i * 128:(i + 1) * 128, :], ob)
    return nc


def build():
    nc = bacc.Bacc("TRN2", target_bir_lowering=False, debug=False)
    emit(nc)
    nc.compile()
    return nc


def host_tables():
    """cosT/sinT [128, 2048] and rmat [128, 128] (same for every core)."""
    d = np.arange(128)
    pair = (d % DH) // 2  # pair index within a head
    inv_freq = THETA ** (-2.0 * pair / DH)  # [128]
    t = np.arange(SEQ, dtype=np.float64)
    ang = t[None, :] * inv_freq[:, None]  # [128, 2048]
    cosT = np.cos(ang).astype(np.float32)
    sinT = np.sin(ang).astype(np.float32)
    R = np.zeros((128, 128), dtype=np.float32)
    idx = np.arange(0, 128, 2)
    R[idx, idx + 1] = -1.0
    R[idx + 1, idx] = 1.0
    rmat = np.ascontiguousarray(R.T)
    return cosT, sinT, rmat


def make_in_maps(x, wq, wk, wv, wo):
    x = np.asarray(x, dtype=np.float32)
    wq = np.asarray(wq, dtype=np.float32)
    wk = np.asarray(wk, dtype=np.float32)
    wv = np.asarray(wv, dtype=np.float32)
    wo = np.asarray(wo, dtype=np.float32)
    cosT, sinT, rmat = host_tables()
    in_maps = []
    for core in range(N_CORES):
        b, hg = core // 4, core % 4
        rows = slice(hg * DQ, (hg + 1) * DQ)
        in_maps.append({
            "xT": np.ascontiguousarray(x[b].T),
            "wqT": np.ascontiguousarray(wq[rows, :].T),
            "wkT": np.ascontiguousarray(wk[rows, :].T),
            "wvT": np.ascontiguousarray(wv[rows, :].T),
            "woT": np.ascontiguousarray(wo[:, rows].T),
            "cosT": cosT,
            "sinT": sinT,
            "rmat": rmat,
        })
    return in_maps


_NC = None
LAST_RESULTS = None


def kernel(x, wq, wk, wv, wo, token_positions=None, **_kwargs):
    """Full-input, full-output entry point. Distributes over 8 NeuronCores."""
    global _NC, LAST_RESULTS
    from concourse import bass_utils

    if _NC is None:
        _NC = build()
    in_maps = make_in_maps(x, wq, wk, wv, wo)
    res = bass_utils.run_bass_kernel_spmd(
        _NC, in_maps, core_ids=list(range(N_CORES))
    )
    LAST_RESULTS = res
    outs = [np.asarray(r["out"]) for r in res.results]
    full = np.empty((BATCH, SEQ, D_MODEL), dtype=np.float32)
    for b in range(BATCH):
        full[b] = outs[4 * b] + outs[4 * b + 1] + outs[4 * b + 2] + outs[4 * b + 3]
    return full
